# revision 7
# baseline (speedup 1.0000x reference)
"""Trainium2 Bass kernel for nn_FFTChainMatrix (block-circulant matmul via 64-pt rFFT).

y = x @ W.T where W is 4096x4096 block-circulant (64x64 grid of 64x64 circulant
blocks) built from channel-weighted circulant_params.  Computed in the FFT
domain, feature-major on device (host handles the token<->feature transposes):

  S1   per 128-feature chunk j (= 2 blocks): X1_j = A.T @ x_j
       A is a permuted block-diagonal rfft64 matrix; its output row order
       (4f + 2c + b) makes every shuffle source a contiguous 4-partition slice.
  shuf X2[fp][(2c+b)*32+j, t] = X1[4fp+2c+b, j-chunk, t]   (SBUF->SBUF DMA)
  S2   per freq-pair fp: Y2[fp] = G[fp].T @ X2[fp]         (complex contraction
       over the 64 in-blocks, embedded as a dense real 128x128)
  unshuf, S3: mirror of shuf/S1 with the irfft matrix B.

Sharding: data-parallel over tokens, 4096 tokens -> 8 cores x 512.
Each stage is 32 matmuls of [128x128] @ [128x256] per token-half; S1/S3 share a
single stationary weight each.  Token-halves pipeline DMA against compute.
"""

from contextlib import ExitStack

import numpy as np

BLK = 64
NCH = 32          # 128-feature chunks per core (= pairs of 64-wide blocks)
T = 512           # tokens per core
TH = 256          # tokens per half
NCORES = 8
FEAT = 4096
COLS = NCH * T    # 16384 columns in the [128, COLS] working tiles


# ---------------------------------------------------------------- host math
def _build_matrices(circulant_params, channel_weights):
    """A (128,128), G (32,128,128), B (128,128) float32 (exact f64 math)."""
    c_w = np.einsum(
        "m,moid->oid",
        np.asarray(channel_weights, np.float64),
        np.asarray(circulant_params, np.float64),
    )
    Chat = np.fft.rfft(c_w, axis=-1)
    Wr, Wi = Chat.real, Chat.imag

    r = np.arange(BLK)
    A64 = np.zeros((64, 64))
    A64[0] = 1.0
    A64[1] = (-1.0) ** r
    B64 = np.zeros((64, 64))
    B64[:, 0] = 1.0 / BLK
    B64[:, 1] = ((-1.0) ** r) / BLK
    for f in range(1, 32):
        A64[2 * f] = np.cos(2 * np.pi * f * r / BLK)
        A64[2 * f + 1] = -np.sin(2 * np.pi * f * r / BLK)
        B64[:, 2 * f] = 2.0 * np.cos(2 * np.pi * f * r / BLK) / BLK
        B64[:, 2 * f + 1] = -2.0 * np.sin(2 * np.pi * f * r / BLK) / BLK

    A = np.zeros((128, 128))
    B = np.zeros((128, 128))
    q2 = 2 * np.arange(64)
    for b in range(2):
        A[b * 64:(b + 1) * 64][:, q2 + b] = A64.T
        B[q2 + b, b * 64:(b + 1) * 64] = B64.T

    i_all = np.arange(64)
    ri = (i_all % 2) * 32 + i_all // 2
    G = np.zeros((32, 128, 128))
    for fp in range(32):
        if fp == 0:
            G[0][np.ix_(ri, ri)] = Wr[:, :, 0].T
            G[0][np.ix_(64 + ri, 64 + ri)] = Wr[:, :, 32].T
        else:
            G[fp][np.ix_(ri, ri)] = Wr[:, :, fp].T
            G[fp][np.ix_(64 + ri, ri)] = -Wi[:, :, fp].T
            G[fp][np.ix_(ri, 64 + ri)] = Wi[:, :, fp].T
            G[fp][np.ix_(64 + ri, 64 + ri)] = Wr[:, :, fp].T
    return A, G, B


# ---------------------------------------------------------------- bass trace
def _trace_nc():
    import concourse.mybir as mybir
    import concourse.tile as tile
    from concourse import bacc

    f32 = mybir.dt.float32
    f16 = mybir.dt.float16

    nc = bacc.Bacc("TRN2", target_bir_lowering=False, debug=False,
                   num_devices=NCORES)
    x_h = nc.dram_tensor("x_shard", [128, COLS], f16, kind="ExternalInput").ap()
    ab_h = nc.dram_tensor("ab_mats", [128, 256], f16, kind="ExternalInput").ap()
    g_h = nc.dram_tensor("g_mats", [128, 32 * 128], f16,
                         kind="ExternalInput").ap()
    y_h = nc.dram_tensor("y_shard", [128, COLS], f16, kind="ExternalOutput").ap()

    cp_ix = [0]
    dma_ix = [0]

    with tile.TileContext(nc) as tc, ExitStack() as ctx:
        wpool = ctx.enter_context(tc.tile_pool(name="weights", bufs=1))
        bigp = ctx.enter_context(tc.tile_pool(name="big", bufs=1))
        y2p = ctx.enter_context(tc.tile_pool(name="y2p", bufs=8))
        mm_ps = ctx.enter_context(tc.tile_pool(name="mm_ps", bufs=7,
                                               space="PSUM"))
        wu_ps = ctx.enter_context(tc.tile_pool(name="wu_ps", bufs=1,
                                               space="PSUM"))

        def copyback(out_ap, in_ap):
            if cp_ix[0] % 2 == 0:
                nc.vector.tensor_copy(out_ap, in_ap)
            else:
                nc.scalar.copy(out_ap, in_ap)
            cp_ix[0] += 1

        def shufdma(dst, src):
            eng = (nc.sync, nc.scalar)[dma_ix[0] % 2]
            dma_ix[0] += 1
            eng.dma_start(dst, src)

        abt = wpool.tile([128, 256], f16)
        nc.sync.dma_start(abt[:], ab_h[:])
        gt = wpool.tile([128, 32 * 128], f16)
        nc.scalar.dma_start(gt[:], g_h[:])

        xa = bigp.tile([128, COLS], f16, tag="xa")
        x1 = bigp.tile([128, COLS], f16, tag="x1")
        x2 = bigp.tile([128, COLS], f16, tag="x2")
        y3 = bigp.tile([128, COLS], f16, tag="y3")
        ya = bigp.tile([128, COLS], f16, tag="ya")

        # ---- input loads: 4 granules of 1 MiB on the SWDGE ring (gpsimd)
        for g in range(4):
            cs = slice(g * 4096, (g + 1) * 4096)
            nc.gpsimd.dma_start(xa[:, cs], x_h[:, cs])

        # ---- PE warm-up while loads run (keeps HAM at 8/8 for S1)
        wu = wu_ps.tile([128, 512], f32, tag="wu")
        for _ in range(10):
            nc.tensor.matmul(wu[:], gt[:, :128], gt[:, 512:1024],
                             start=True, stop=True)

        at = abt[:, 0:128]
        bt = abt[:, 128:256]

        def s1(h):
            for jp in range(NCH // 2):          # chunk pairs
                ps = mm_ps.tile([128, 512], f32, tag="mm")
                for jl in range(2):
                    j = 2 * jp + jl
                    c = slice(h * 8192 + j * TH, h * 8192 + (j + 1) * TH)
                    nc.tensor.matmul(ps[:, jl * TH:(jl + 1) * TH],
                                     at, xa[:, c], start=True, stop=True)
                oc = slice(h * 8192 + 2 * jp * TH, h * 8192 + (2 * jp + 2) * TH)
                copyback(x1[:, oc], ps[:])

        def shuffle(h):
            for fp in range(NCH):               # freq pairs
                src = x1[4 * fp:4 * fp + 4,
                         h * 8192:(h + 1) * 8192].rearrange(
                    "p (j t) -> p j t", t=TH)
                dst = x2[:, h * 8192 + fp * TH: h * 8192 + (fp + 1) * TH]
                shufdma(dst, src)

        def s2_unshuffle(h):
            for fg in range(16):
                ps = mm_ps.tile([128, 512], f32, tag="mm")
                for fpl in range(2):
                    fp = 2 * fg + fpl
                    c = slice(h * 8192 + fp * TH, h * 8192 + (fp + 1) * TH)
                    nc.tensor.matmul(ps[:, fpl * TH:(fpl + 1) * TH],
                                     gt[:, fp * 128:(fp + 1) * 128],
                                     x2[:, c], start=True, stop=True)
                y2c = y2p.tile([128, 512], f16, tag="y2")
                copyback(y2c[:], ps[:])
                for fpl in range(2):
                    fp = 2 * fg + fpl
                    dst = y3[4 * fp:4 * fp + 4,
                             h * 8192:(h + 1) * 8192].rearrange(
                        "p (j t) -> p j t", t=TH)
                    shufdma(dst, y2c[:, fpl * TH:(fpl + 1) * TH])

        def s3(h):
            for jp in range(NCH // 2):
                ps = mm_ps.tile([128, 512], f32, tag="mm")
                for jl in range(2):
                    j = 2 * jp + jl
                    c = slice(h * 8192 + j * TH, h * 8192 + (j + 1) * TH)
                    nc.tensor.matmul(ps[:, jl * TH:(jl + 1) * TH],
                                     bt, y3[:, c], start=True, stop=True)
                oc = slice(h * 8192 + 2 * jp * TH, h * 8192 + (2 * jp + 2) * TH)
                copyback(ya[:, oc], ps[:])

        def store(h):
            for q in range(2):
                cs = slice(h * 8192 + q * 4096, h * 8192 + (q + 1) * 4096)
                shufdma(y_h[:, cs], ya[:, cs])

        # software-pipelined emission over token halves
        s1(0)
        shuffle(0)
        s1(1)
        s2_unshuffle(0)
        shuffle(1)
        s3(0)
        store(0)
        s2_unshuffle(1)
        s3(1)
        store(1)

    nc.compile()
    return nc


_CACHE = {}


def make_in_maps(x, circulant_params, channel_weights):
    xf = np.ascontiguousarray(np.asarray(x, np.float32)).reshape(-1, FEAT)
    assert xf.shape[0] == NCORES * T, f"unexpected token count {xf.shape}"
    A, G, B = _build_matrices(circulant_params, channel_weights)
    ab = np.concatenate([A, B], axis=1).astype(np.float16)
    g_kfm = np.ascontiguousarray(
        G.transpose(1, 0, 2).reshape(128, 32 * 128).astype(np.float16))
    xf16 = xf.astype(np.float16)
    in_maps = []
    for c in range(NCORES):
        xc = xf16[c * T:(c + 1) * T]                       # [512, 4096]
        # x_shard[p, h*8192 + j*256 + t] = xc[h*256+t, 128j+p]
        xs = np.ascontiguousarray(
            xc.reshape(2, TH, NCH, 128).transpose(3, 0, 2, 1).reshape(128, COLS))
        in_maps.append({"x_shard": xs, "ab_mats": ab, "g_mats": g_kfm})
    return in_maps


def kernel(x, circulant_params, channel_weights):
    from concourse.bass_utils import run_bass_kernel_spmd

    x = np.ascontiguousarray(np.asarray(x, np.float32))
    orig_shape = x.shape

    if "nc" not in _CACHE:
        _CACHE["nc"] = _trace_nc()
    nc = _CACHE["nc"]

    in_maps = make_in_maps(x, circulant_params, channel_weights)
    res = run_bass_kernel_spmd(nc, in_maps, core_ids=list(range(NCORES)))
    outs = []
    for c in range(NCORES):
        ys = res.results[c]["y_shard"]                     # [128, COLS]
        # y[h*256+t, 128j+p] = ys[p, h*8192 + j*256 + t]
        yc = ys.reshape(128, 2, NCH, TH).transpose(1, 3, 2, 0).reshape(T, FEAT)
        outs.append(yc)
    y = np.concatenate(outs, axis=0)
    return y.astype(np.float32).reshape(orig_shape)


# revision 10
# speedup vs baseline: 1.8801x; 1.8801x over previous
"""Trainium2 Bass kernel for nn_FFTChainMatrix (block-circulant matmul via 64-pt rFFT).

y = x @ W.T where W is 4096x4096 block-circulant (64x64 grid of 64x64 circulant
blocks) built from channel-weighted circulant_params.  Computed in the FFT
domain.  The two partition<->column exchanges between FFT stages run on the
DMA XBAR (dma_start_transpose, blockwise 128x128, serialized on one HWDGE
ring), which measured ~194 GB/s vs ~68 GB/s for partition-gather DMAs:

  S1T  per (chunk j, 128-token tile): PS = x_j.T @ A    -> [tok, freqrow] in
       PSUM; the PSUM->SBUF copyback scatters columns into X1T so that each
       128-column block fp holds that freq-pair's rows for all chunks.
  Xxbar  one transpose-DMA per token-tile: X2 = blockwise-T(X1T)
         -> [freq-pair-major partitions, tokens]
  S2T  per freq-pair fp: PS = X2[fp].T @ G[fp]  (complex contraction over the
       64 in-blocks); copyback scatters into Y2T grouped by out-chunk.
  Yxbar  transpose back -> Y3 [per-out-chunk freq rows, tokens]
  S3   per out-chunk: B.T @ Y3_j -> y (feature-major), DMA to HBM.

Sharding: data-parallel over tokens, 4096 tokens -> 8 cores x 512.
Token-halves pipeline HBM I/O and PE against the serialized XBAR chain.
"""

from contextlib import ExitStack

import numpy as np

BLK = 64
NCH = 32          # 128-feature chunks per core (= pairs of 64-wide blocks)
T = 512           # tokens per core
TH = 256          # tokens per half
NCORES = 8
FEAT = 4096
COLS = NCH * T    # 16384 columns in the [128, COLS] working tiles


# ---------------------------------------------------------------- host math
def _build_matrices(circulant_params, channel_weights):
    """A (128,128), G (32,128,128), B (128,128) float64.

    A cols: 4f+2c+b  (freq f, re/im c, block-in-pair b) applied per chunk.
    G rows: c*64+b*32+j (in-block i=2j+b);  G cols: j'*4+2c'+b' (out-block
    o=2j'+b') -- the column order lands S2T output directly in Y2T order.
    B rows: 4f+2c'+b';  B cols: b'*64+d.
    """
    c_w = np.einsum(
        "m,moid->oid",
        np.asarray(channel_weights, np.float64),
        np.asarray(circulant_params, np.float64),
    )
    Chat = np.fft.rfft(c_w, axis=-1)
    Wr, Wi = Chat.real, Chat.imag

    r = np.arange(BLK)
    A64 = np.zeros((64, 64))
    A64[0] = 1.0
    A64[1] = (-1.0) ** r
    B64 = np.zeros((64, 64))
    B64[:, 0] = 1.0 / BLK
    B64[:, 1] = ((-1.0) ** r) / BLK
    for f in range(1, 32):
        A64[2 * f] = np.cos(2 * np.pi * f * r / BLK)
        A64[2 * f + 1] = -np.sin(2 * np.pi * f * r / BLK)
        B64[:, 2 * f] = 2.0 * np.cos(2 * np.pi * f * r / BLK) / BLK
        B64[:, 2 * f + 1] = -2.0 * np.sin(2 * np.pi * f * r / BLK) / BLK

    A = np.zeros((128, 128))
    B = np.zeros((128, 128))
    q2 = 2 * np.arange(64)
    for b in range(2):
        A[b * 64:(b + 1) * 64][:, q2 + b] = A64.T
        B[q2 + b, b * 64:(b + 1) * 64] = B64.T

    i_all = np.arange(64)
    ri = (i_all % 2) * 32 + i_all // 2
    G = np.zeros((32, 128, 128))
    for fp in range(32):
        if fp == 0:
            G[0][np.ix_(ri, ri)] = Wr[:, :, 0].T
            G[0][np.ix_(64 + ri, 64 + ri)] = Wr[:, :, 32].T
        else:
            G[fp][np.ix_(ri, ri)] = Wr[:, :, fp].T
            G[fp][np.ix_(64 + ri, ri)] = -Wi[:, :, fp].T
            G[fp][np.ix_(ri, 64 + ri)] = Wi[:, :, fp].T
            G[fp][np.ix_(64 + ri, 64 + ri)] = Wr[:, :, fp].T
    # reorder G cols: old c*64+b*32+j  ->  new j*4+2c+b
    perm = np.zeros(128, np.int64)
    for c in range(2):
        for b in range(2):
            for j in range(32):
                perm[j * 4 + 2 * c + b] = c * 64 + b * 32 + j
    G = G[:, :, perm]
    return A, G, B


# ---------------------------------------------------------------- bass trace
def _trace_nc():
    import concourse.mybir as mybir
    import concourse.tile as tile
    from concourse import bacc

    f32 = mybir.dt.float32
    f16 = mybir.dt.float16

    nc = bacc.Bacc("TRN2", target_bir_lowering=False, debug=False,
                   num_devices=NCORES)
    x_h = nc.dram_tensor("x_shard", [128, COLS], f16, kind="ExternalInput").ap()
    a_h = nc.dram_tensor("a_mats", [128, 128], f16, kind="ExternalInput").ap()
    b_h = nc.dram_tensor("b_mats", [128, 128], f16, kind="ExternalInput").ap()
    g_h = nc.dram_tensor("g_mats", [128, 32 * 128], f16,
                         kind="ExternalInput").ap()
    y_h = nc.dram_tensor("y_shard", [128, COLS], f16, kind="ExternalOutput").ap()

    cp_ix = [0]

    with tile.TileContext(nc) as tc, ExitStack() as ctx:
        wpool = ctx.enter_context(tc.tile_pool(name="weights", bufs=1))
        bigp = ctx.enter_context(tc.tile_pool(name="big", bufs=1))
        mm_ps = ctx.enter_context(tc.tile_pool(name="mm_ps", bufs=7,
                                               space="PSUM"))
        wu_ps = ctx.enter_context(tc.tile_pool(name="wu_ps", bufs=1,
                                               space="PSUM"))

        def copyback(out_ap, in_ap):
            if cp_ix[0] % 2 == 0:
                nc.vector.tensor_copy(out_ap, in_ap)
            else:
                nc.scalar.copy(out_ap, in_ap)
            cp_ix[0] += 1

        at = wpool.tile([128, 128], f16)
        nc.scalar.dma_start(at[:], a_h[:])
        gt = wpool.tile([128, 32 * 128], f16)
        nc.scalar.dma_start(gt[:], g_h[:])
        bt = wpool.tile([128, 128], f16)
        nc.scalar.dma_start(bt[:], b_h[:])

        xa = bigp.tile([128, COLS], f16, tag="xa")   # also reused as ya
        x2 = bigp.tile([128, COLS], f16, tag="x2")
        y3 = bigp.tile([128, COLS], f16, tag="y3")
        x1t = [[bigp.tile([128, 4096], f16, tag=f"x1t{h}{tq}",
                          name=f"x1t{h}{tq}")
                for tq in range(2)] for h in range(2)]
        y2t = [[bigp.tile([128, 4096], f16, tag=f"y2t{h}{tq}",
                          name=f"y2t{h}{tq}")
                for tq in range(2)] for h in range(2)]

        # ---- input loads on the SWDGE ring (frees both HWDGE rings)
        for g in range(4):
            cs = slice(g * 4096, (g + 1) * 4096)
            nc.gpsimd.dma_start(xa[:, cs], x_h[:, cs])

        # ---- PE warm-up while loads run
        wu = wu_ps.tile([128, 512], f32, tag="wu")
        for _ in range(10):
            nc.tensor.matmul(wu[:, :128], at[:], at[:, :128],
                             start=True, stop=True)

        def s1t(h):
            for tq in range(2):
                dst4 = x1t[h][tq][:].rearrange(
                    "p (f cb j) -> p f cb j", cb=4, j=32)
                for jg in range(8):
                    ps = mm_ps.tile([128, 512], f32, tag="mm")
                    for jl in range(4):
                        j = 4 * jg + jl
                        c0 = h * 8192 + j * 256 + tq * 128
                        nc.tensor.matmul(ps[:, jl * 128:(jl + 1) * 128],
                                         xa[:, c0:c0 + 128], at[:],
                                         start=True, stop=True)
                    src = ps[:].rearrange(
                        "p (jl f cb) -> p f cb jl", f=32, cb=4)
                    copyback(dst4[:, :, :, 4 * jg:4 * jg + 4], src)

        def xxbar(h):
            for tq in range(2):
                out = x2[:, h * 8192 + tq * 4096:
                         h * 8192 + (tq + 1) * 4096].rearrange(
                    "p (k m) -> p k m", m=128)
                nc.sync.dma_start_transpose(out, x1t[h][tq][:])

        def s2t(h):
            for tq in range(2):
                dst4 = y2t[h][tq][:].rearrange(
                    "p (jp f cbp) -> p jp f cbp", f=32, cbp=4)
                for fg in range(8):
                    ps = mm_ps.tile([128, 512], f32, tag="mm")
                    for fl in range(4):
                        fp = 4 * fg + fl
                        c0 = h * 8192 + tq * 4096 + fp * 128
                        nc.tensor.matmul(ps[:, fl * 128:(fl + 1) * 128],
                                         x2[:, c0:c0 + 128],
                                         gt[:, fp * 128:(fp + 1) * 128],
                                         start=True, stop=True)
                    src = ps[:].rearrange(
                        "p (fl jp cbp) -> p jp fl cbp", jp=32, cbp=4)
                    copyback(dst4[:, :, 4 * fg:4 * fg + 4, :], src)

        def yxbar(h):
            for tq in range(2):
                out = y3[:, h * 8192 + tq * 4096:
                         h * 8192 + (tq + 1) * 4096].rearrange(
                    "p (k m) -> p k m", m=128)
                nc.sync.dma_start_transpose(out, y2t[h][tq][:])

        def s3(h):
            rhs_h = y3[:, h * 8192:(h + 1) * 8192].rearrange(
                "p (tq jp m) -> p jp tq m", tq=2, m=128)
            for jp2 in range(16):
                ps = mm_ps.tile([128, 512], f32, tag="mm")
                for jl in range(2):
                    jp = 2 * jp2 + jl
                    nc.tensor.matmul(ps[:, jl * 256:(jl + 1) * 256],
                                     bt[:], rhs_h[:, jp],
                                     start=True, stop=True)
                oc = slice(h * 8192 + 2 * jp2 * 256, h * 8192 + (2 * jp2 + 2) * 256)
                copyback(xa[:, oc], ps[:])   # xa reused as ya

        def store(h):
            for q in range(2):
                cs = slice(h * 8192 + q * 4096, h * 8192 + (q + 1) * 4096)
                nc.scalar.dma_start(y_h[:, cs], xa[:, cs])

        # software-pipelined emission: PE order S1T0,S1T1,S2T0,S3-0,S2T1,S3-1
        # xbar ring order Xx0,Yx0,Xx1,Yx1 (all on nc.sync, serialized)
        s1t(0)
        xxbar(0)
        s1t(1)
        s2t(0)
        yxbar(0)
        xxbar(1)
        s3(0)
        store(0)
        s2t(1)
        yxbar(1)
        s3(1)
        store(1)

    nc.compile()
    return nc


_CACHE = {}


def make_in_maps(x, circulant_params, channel_weights):
    xf = np.ascontiguousarray(np.asarray(x, np.float32)).reshape(-1, FEAT)
    assert xf.shape[0] == NCORES * T, f"unexpected token count {xf.shape}"
    A, G, B = _build_matrices(circulant_params, channel_weights)
    a16 = A.astype(np.float16)
    b16 = B.astype(np.float16)
    g_kfm = np.ascontiguousarray(
        G.transpose(1, 0, 2).reshape(128, 32 * 128).astype(np.float16))
    xf16 = xf.astype(np.float16)
    in_maps = []
    for c in range(NCORES):
        xc = xf16[c * T:(c + 1) * T]                       # [512, 4096]
        # x_shard[p, h*8192 + j*256 + t] = xc[h*256+t, 128j+p]
        xs = np.ascontiguousarray(
            xc.reshape(2, TH, NCH, 128).transpose(3, 0, 2, 1).reshape(128, COLS))
        in_maps.append({"x_shard": xs, "a_mats": a16, "b_mats": b16,
                        "g_mats": g_kfm})
    return in_maps


def kernel(x, circulant_params, channel_weights):
    from concourse.bass_utils import run_bass_kernel_spmd

    x = np.ascontiguousarray(np.asarray(x, np.float32))
    orig_shape = x.shape

    if "nc" not in _CACHE:
        _CACHE["nc"] = _trace_nc()
    nc = _CACHE["nc"]

    in_maps = make_in_maps(x, circulant_params, channel_weights)
    res = run_bass_kernel_spmd(nc, in_maps, core_ids=list(range(NCORES)))
    outs = []
    for c in range(NCORES):
        ys = res.results[c]["y_shard"]                     # [128, COLS]
        # y[h*256+t, 128j+p] = ys[p, h*8192 + j*256 + t]
        yc = ys.reshape(128, 2, NCH, TH).transpose(1, 3, 2, 0).reshape(T, FEAT)
        outs.append(yc)
    y = np.concatenate(outs, axis=0)
    return y.astype(np.float32).reshape(orig_shape)


# revision 13
# speedup vs baseline: 1.9768x; 1.0514x over previous
"""Trainium2 Bass kernel for nn_FFTChainMatrix (block-circulant matmul via 64-pt rFFT).

y = x @ W.T where W is 4096x4096 block-circulant (64x64 grid of 64x64 circulant
blocks) built from channel-weighted circulant_params.  Computed in the FFT
domain.  The two partition<->column exchanges between FFT stages run on the
DMA XBAR (dma_start_transpose, blockwise 128x128, serialized on one HWDGE
ring), which measured ~194 GB/s vs ~68 GB/s for partition-gather DMAs:

  S1T  per (chunk j, 128-token tile): PS = x_j.T @ A    -> [tok, freqrow] in
       PSUM; the PSUM->SBUF copyback scatters columns into X1T so that each
       128-column block fp holds that freq-pair's rows for all chunks.
  Xxbar  one transpose-DMA per token-tile: X2 = blockwise-T(X1T)
         -> [freq-pair-major partitions, tokens]
  S2T  per freq-pair fp: PS = X2[fp].T @ G[fp]  (complex contraction over the
       64 in-blocks); copyback scatters into Y2T grouped by out-chunk.
  Yxbar  transpose back -> Y3 [per-out-chunk freq rows, tokens]
  S3   per out-chunk: B.T @ Y3_j -> y (feature-major), DMA to HBM.

Sharding: data-parallel over tokens, 4096 tokens -> 8 cores x 512.
Token-halves pipeline HBM I/O and PE against the serialized XBAR chain.
"""

from contextlib import ExitStack

import numpy as np

BLK = 64
NCH = 32          # 128-feature chunks per core (= pairs of 64-wide blocks)
T = 512           # tokens per core
TH = 256          # tokens per half
NCORES = 8
FEAT = 4096
COLS = NCH * T    # 16384 columns in the [128, COLS] working tiles


# ---------------------------------------------------------------- host math
def _build_matrices(circulant_params, channel_weights):
    """A (128,128), G (32,128,128), B (128,128) float64.

    A cols: 4f+2c+b  (freq f, re/im c, block-in-pair b) applied per chunk.
    G rows: c*64+b*32+j (in-block i=2j+b);  G cols: j'*4+2c'+b' (out-block
    o=2j'+b') -- the column order lands S2T output directly in Y2T order.
    B rows: 4f+2c'+b';  B cols: b'*64+d.
    """
    c_w = np.einsum(
        "m,moid->oid",
        np.asarray(channel_weights, np.float64),
        np.asarray(circulant_params, np.float64),
    )
    Chat = np.fft.rfft(c_w, axis=-1)
    Wr, Wi = Chat.real, Chat.imag

    r = np.arange(BLK)
    A64 = np.zeros((64, 64))
    A64[0] = 1.0
    A64[1] = (-1.0) ** r
    B64 = np.zeros((64, 64))
    B64[:, 0] = 1.0 / BLK
    B64[:, 1] = ((-1.0) ** r) / BLK
    for f in range(1, 32):
        A64[2 * f] = np.cos(2 * np.pi * f * r / BLK)
        A64[2 * f + 1] = -np.sin(2 * np.pi * f * r / BLK)
        B64[:, 2 * f] = 2.0 * np.cos(2 * np.pi * f * r / BLK) / BLK
        B64[:, 2 * f + 1] = -2.0 * np.sin(2 * np.pi * f * r / BLK) / BLK

    A = np.zeros((128, 128))
    B = np.zeros((128, 128))
    q2 = 2 * np.arange(64)
    for b in range(2):
        A[b * 64:(b + 1) * 64][:, q2 + b] = A64.T
        B[q2 + b, b * 64:(b + 1) * 64] = B64.T

    i_all = np.arange(64)
    ri = (i_all % 2) * 32 + i_all // 2
    G = np.zeros((32, 128, 128))
    for fp in range(32):
        if fp == 0:
            G[0][np.ix_(ri, ri)] = Wr[:, :, 0].T
            G[0][np.ix_(64 + ri, 64 + ri)] = Wr[:, :, 32].T
        else:
            G[fp][np.ix_(ri, ri)] = Wr[:, :, fp].T
            G[fp][np.ix_(64 + ri, ri)] = -Wi[:, :, fp].T
            G[fp][np.ix_(ri, 64 + ri)] = Wi[:, :, fp].T
            G[fp][np.ix_(64 + ri, 64 + ri)] = Wr[:, :, fp].T
    # reorder G cols: old c*64+b*32+j  ->  new j*4+2c+b
    perm = np.zeros(128, np.int64)
    for c in range(2):
        for b in range(2):
            for j in range(32):
                perm[j * 4 + 2 * c + b] = c * 64 + b * 32 + j
    G = G[:, :, perm]
    return A, G, B


# ---------------------------------------------------------------- bass trace
def _trace_nc():
    import concourse.mybir as mybir
    import concourse.tile as tile
    from concourse import bacc

    f32 = mybir.dt.float32
    f16 = mybir.dt.float16

    nc = bacc.Bacc("TRN2", target_bir_lowering=False, debug=False,
                   num_devices=NCORES)
    x_h = nc.dram_tensor("x_shard", [128, COLS], f16, kind="ExternalInput").ap()
    a_h = nc.dram_tensor("a_mats", [128, 128], f16, kind="ExternalInput").ap()
    b_h = nc.dram_tensor("b_mats", [128, 128], f16, kind="ExternalInput").ap()
    g_h = nc.dram_tensor("g_mats", [128, 32 * 128], f16,
                         kind="ExternalInput").ap()
    y_h = nc.dram_tensor("y_shard", [128, COLS], f16, kind="ExternalOutput").ap()

    cp_ix = [0]

    with tile.TileContext(nc) as tc, ExitStack() as ctx:
        wpool = ctx.enter_context(tc.tile_pool(name="weights", bufs=1))
        bigp = ctx.enter_context(tc.tile_pool(name="big", bufs=1))
        mm_ps = ctx.enter_context(tc.tile_pool(name="mm_ps", bufs=7,
                                               space="PSUM"))
        wu_ps = ctx.enter_context(tc.tile_pool(name="wu_ps", bufs=1,
                                               space="PSUM"))

        def copyback(out_ap, in_ap):
            if cp_ix[0] % 2 == 0:
                nc.vector.tensor_copy(out_ap, in_ap)
            else:
                nc.scalar.copy(out_ap, in_ap)
            cp_ix[0] += 1

        at = wpool.tile([128, 128], f16)
        bt = wpool.tile([128, 128], f16)
        gt = wpool.tile([128, 32 * 128], f16)

        xa = bigp.tile([128, COLS], f16, tag="xa")   # also reused as ya
        x2 = bigp.tile([128, COLS], f16, tag="x2")
        y3 = bigp.tile([128, COLS], f16, tag="y3")
        x1t = [[bigp.tile([128, 4096], f16, tag=f"x1t{h}{tq}",
                          name=f"x1t{h}{tq}")
                for tq in range(2)] for h in range(2)]
        y2t = [[bigp.tile([128, 4096], f16, tag=f"y2t{h}{tq}",
                          name=f"y2t{h}{tq}")
                for tq in range(2)] for h in range(2)]

        # ---- input loads: first granule on sync (fast HWDGE start, before
        # the xbar chain), one on scalar after the small A matrix, the rest
        # on the SWDGE ring
        nc.sync.dma_start(xa[:, 0:4096], x_h[:, 0:4096])
        nc.scalar.dma_start(at[:], a_h[:])
        nc.scalar.dma_start(xa[:, 4096:8192], x_h[:, 4096:8192])
        nc.gpsimd.dma_start(xa[:, 8192:12288], x_h[:, 8192:12288])
        nc.gpsimd.dma_start(xa[:, 12288:16384], x_h[:, 12288:16384])
        nc.scalar.dma_start(gt[:], g_h[:])
        nc.scalar.dma_start(bt[:], b_h[:])

        # ---- PE warm-up while loads run
        wu = wu_ps.tile([128, 512], f32, tag="wu")
        for _ in range(10):
            nc.tensor.matmul(wu[:, :128], at[:], at[:, :128],
                             start=True, stop=True)

        def s1t(h):
            for tq in range(2):
                dst4 = x1t[h][tq][:].rearrange(
                    "p (f cb j) -> p f cb j", cb=4, j=32)
                for jg in range(8):
                    ps = mm_ps.tile([128, 512], f32, tag="mm")
                    for jl in range(4):
                        j = 4 * jg + jl
                        c0 = h * 8192 + j * 256 + tq * 128
                        nc.tensor.matmul(ps[:, jl * 128:(jl + 1) * 128],
                                         xa[:, c0:c0 + 128], at[:],
                                         start=True, stop=True)
                    src = ps[:].rearrange(
                        "p (jl f cb) -> p f cb jl", f=32, cb=4)
                    copyback(dst4[:, :, :, 4 * jg:4 * jg + 4], src)

        def xxbar(h):
            for tq in range(2):
                out = x2[:, h * 8192 + tq * 4096:
                         h * 8192 + (tq + 1) * 4096].rearrange(
                    "p (k m) -> p k m", m=128)
                nc.sync.dma_start_transpose(out, x1t[h][tq][:])

        def s2t(h):
            for tq in range(2):
                dst4 = y2t[h][tq][:].rearrange(
                    "p (jp f cbp) -> p jp f cbp", f=32, cbp=4)
                for fg in range(8):
                    ps = mm_ps.tile([128, 512], f32, tag="mm")
                    for fl in range(4):
                        fp = 4 * fg + fl
                        c0 = h * 8192 + tq * 4096 + fp * 128
                        nc.tensor.matmul(ps[:, fl * 128:(fl + 1) * 128],
                                         x2[:, c0:c0 + 128],
                                         gt[:, fp * 128:(fp + 1) * 128],
                                         start=True, stop=True)
                    src = ps[:].rearrange(
                        "p (fl jp cbp) -> p jp fl cbp", jp=32, cbp=4)
                    copyback(dst4[:, :, 4 * fg:4 * fg + 4, :], src)

        def yxbar(h):
            for tq in range(2):
                out = y3[:, h * 8192 + tq * 4096:
                         h * 8192 + (tq + 1) * 4096].rearrange(
                    "p (k m) -> p k m", m=128)
                nc.sync.dma_start_transpose(out, y2t[h][tq][:])

        def s3(h):
            rhs_h = y3[:, h * 8192:(h + 1) * 8192].rearrange(
                "p (tq jp m) -> p jp tq m", tq=2, m=128)
            for jp2 in range(16):
                ps = mm_ps.tile([128, 512], f32, tag="mm")
                for jl in range(2):
                    jp = 2 * jp2 + jl
                    nc.tensor.matmul(ps[:, jl * 256:(jl + 1) * 256],
                                     bt[:], rhs_h[:, jp],
                                     start=True, stop=True)
                oc = slice(h * 8192 + 2 * jp2 * 256, h * 8192 + (2 * jp2 + 2) * 256)
                copyback(xa[:, oc], ps[:])   # xa reused as ya

        def store(h):
            for q in range(2):
                cs = slice(h * 8192 + q * 4096, h * 8192 + (q + 1) * 4096)
                nc.scalar.dma_start(y_h[:, cs], xa[:, cs])

        # software-pipelined emission: PE order S1T0,S1T1,S2T0,S2T1,S3-0,S3-1
        # xbar ring order Xx0,Xx1,Yx0,Yx1 (all on nc.sync, serialized) packs
        # the chain gap-free: each hop's producer runs during the prior hop
        s1t(0)
        s1t(1)
        xxbar(0)
        xxbar(1)
        s2t(0)
        s2t(1)
        yxbar(0)
        yxbar(1)
        s3(0)
        store(0)
        s3(1)
        store(1)

    nc.compile()
    return nc


_CACHE = {}


def make_in_maps(x, circulant_params, channel_weights):
    xf = np.ascontiguousarray(np.asarray(x, np.float32)).reshape(-1, FEAT)
    assert xf.shape[0] == NCORES * T, f"unexpected token count {xf.shape}"
    A, G, B = _build_matrices(circulant_params, channel_weights)
    a16 = A.astype(np.float16)
    b16 = B.astype(np.float16)
    g_kfm = np.ascontiguousarray(
        G.transpose(1, 0, 2).reshape(128, 32 * 128).astype(np.float16))
    xf16 = xf.astype(np.float16)
    in_maps = []
    for c in range(NCORES):
        xc = xf16[c * T:(c + 1) * T]                       # [512, 4096]
        # x_shard[p, h*8192 + j*256 + t] = xc[h*256+t, 128j+p]
        xs = np.ascontiguousarray(
            xc.reshape(2, TH, NCH, 128).transpose(3, 0, 2, 1).reshape(128, COLS))
        in_maps.append({"x_shard": xs, "a_mats": a16, "b_mats": b16,
                        "g_mats": g_kfm})
    return in_maps


def kernel(x, circulant_params, channel_weights):
    from concourse.bass_utils import run_bass_kernel_spmd

    x = np.ascontiguousarray(np.asarray(x, np.float32))
    orig_shape = x.shape

    if "nc" not in _CACHE:
        _CACHE["nc"] = _trace_nc()
    nc = _CACHE["nc"]

    in_maps = make_in_maps(x, circulant_params, channel_weights)
    res = run_bass_kernel_spmd(nc, in_maps, core_ids=list(range(NCORES)))
    outs = []
    for c in range(NCORES):
        ys = res.results[c]["y_shard"]                     # [128, COLS]
        # y[h*256+t, 128j+p] = ys[p, h*8192 + j*256 + t]
        yc = ys.reshape(128, 2, NCH, TH).transpose(1, 3, 2, 0).reshape(T, FEAT)
        outs.append(yc)
    y = np.concatenate(outs, axis=0)
    return y.astype(np.float32).reshape(orig_shape)
